# revision 1
# baseline (speedup 1.0000x reference)
"""Trainium2 Bass kernel: multi-head attention with RoPE (causal), 8-core SPMD.

Sharding: 8 cores = 4 batches x 2 head-halves (tensor parallel over heads,
data parallel over batch). Each core computes, for its batch b and its 8
heads: QKV projections, RoPE, causal attention, and a partial output
projection. Host sums the two head-half partials per batch and adds the bias.

All TensorEngine math in bf16 with fp32 PSUM accumulation.

Schedule: attention(h) is interleaved with produce(h+1) at qc/block
granularity so the Act engine's exp work (which saturates a pure attention
window) spreads across the projection windows. Phase-3 (output projection)
first column group is interleaved into attention(7) the same way.

DMA queues: xt chunks on scalar; weights + consts + y write-back on sync;
RoPE swap SBUF-SBUF copies on gpsimd.
"""

import sys

for _p in ("/opt/trn_rl_repo",):
    if _p not in sys.path:
        sys.path.insert(0, _p)

import numpy as np
import ml_dtypes

import concourse.bass as bass
import concourse.bacc as bacc
import concourse.mybir as mybir
import concourse.tile as tile
from concourse.bass_utils import run_bass_kernel_spmd

BF16 = ml_dtypes.bfloat16

B, T, C = 4, 2048, 2048
H = 16
D = C // H  # 128
ROPE_THETA = 1000000.0
N_CORES = 8
HPC = H // 2          # heads per core (8)
P = 128               # partitions
CHUNK = 512           # moving free dim per matmul
N_CC = C // P         # 16 contraction chunks
N_QC = T // CHUNK     # 4 q-chunks
N_KT = T // P         # 16 k-tiles
N_TT = T // P         # 16 t-tiles
SCALE = 1.0 / float(np.sqrt(D))

_CACHED = {}


def build_kernel():
    """Build the SPMD Bass program (identical on all 8 cores)."""
    fp32 = mybir.dt.float32
    bf16 = mybir.dt.bfloat16

    nc = bacc.Bacc("TRN2", target_bir_lowering=False, debug=False,
                   num_devices=N_CORES)

    # Per-core DRAM inputs (bf16 unless noted)
    xt = nc.dram_tensor("xt", [C, T], bf16, kind="ExternalInput")          # x[b].T
    wq = nc.dram_tensor("wq", [HPC, C, D], bf16, kind="ExternalInput")     # Wq_h.T (perm'd)
    wk = nc.dram_tensor("wk", [HPC, C, D], bf16, kind="ExternalInput")
    wv = nc.dram_tensor("wv", [HPC // 2, C, 2 * D], bf16, kind="ExternalInput")  # head pairs
    wo = nc.dram_tensor("wo", [HPC * D, C], bf16, kind="ExternalInput")    # WoT slice
    cs = nc.dram_tensor("cs", [P, T], bf16, kind="ExternalInput")          # [cosT;sinT]
    sc = nc.dram_tensor("sc", [P, T], bf16, kind="ExternalInput")          # [sinT;cosT]
    tri = nc.dram_tensor("tri", [P, P], bf16, kind="ExternalInput")        # k<=q mask
    y = nc.dram_tensor("y", [T, C], bf16, kind="ExternalOutput")

    Exp = mybir.ActivationFunctionType.Exp

    with tile.TileContext(nc) as tc:
        with (
            tc.tile_pool(name="const", bufs=1) as const_pool,
            tc.tile_pool(name="xtp", bufs=1) as xt_pool,
            tc.tile_pool(name="wqk", bufs=4) as wqk_pool,
            tc.tile_pool(name="wvp", bufs=2) as wv_pool,
            tc.tile_pool(name="vw", bufs=4) as vw_pool,
            tc.tile_pool(name="swp", bufs=4) as swp_pool,
            tc.tile_pool(name="qktr", bufs=2) as qk_pool,
            tc.tile_pool(name="vsb", bufs=2) as v_pool,
            tc.tile_pool(name="pt", bufs=3) as pt_pool,
            tc.tile_pool(name="rcp", bufs=2) as rcp_pool,
            tc.tile_pool(name="aot", bufs=1) as aot_pool,
            tc.tile_pool(name="wop", bufs=16) as wo_pool,
            tc.tile_pool(name="yst", bufs=3) as y_pool,
            tc.tile_pool(name="ph1", bufs=2, space="PSUM") as ph1_psum,
            tc.tile_pool(name="pst", bufs=2, space="PSUM") as s_psum,
            tc.tile_pool(name="po", bufs=1, space="PSUM") as o_psum,
            tc.tile_pool(name="pr", bufs=1, space="PSUM") as r_psum,
        ):
            # ---- weight loads: dedicated sync queue, prefetched ahead ----
            qk_w = {}

            def load_qk_w(h):
                wq_sb = wqk_pool.tile([P, N_CC, D], bf16, tag="wqk")
                nc.sync.dma_start(
                    out=wq_sb, in_=wq.ap()[h].rearrange("(cc p) d -> p cc d", p=P))
                wk_sb = wqk_pool.tile([P, N_CC, D], bf16, tag="wqk")
                nc.sync.dma_start(
                    out=wk_sb, in_=wk.ap()[h].rearrange("(cc p) d -> p cc d", p=P))
                qk_w[h] = (wq_sb, wk_sb)

            wv_w = {}

            def load_wv(pair):
                w_sb = wv_pool.tile([P, N_CC, 2 * D], bf16, tag="wv")
                nc.sync.dma_start(
                    out=w_sb, in_=wv.ap()[pair].rearrange("(cc p) d -> p cc d", p=P))
                wv_w[pair] = w_sb

            load_qk_w(0)

            # ---- constants (sync queue, right after head-0 weights) ----
            cs_sb = const_pool.tile([P, T], bf16)
            sc_sb = const_pool.tile([P, T], bf16)
            tri_sb = const_pool.tile([P, P], bf16)
            ones_sb = const_pool.tile([P, P], bf16)
            nc.sync.dma_start(out=cs_sb, in_=cs.ap())
            nc.sync.dma_start(out=sc_sb, in_=sc.ap())
            load_wv(0)  # ahead of the xt stream on the sync queue
            nc.sync.dma_start(out=tri_sb, in_=tri.ap())
            nc.vector.memset(ones_sb, 1.0)

            # ---- x^T load: 64 chunk tiles ----
            xt_t = {}
            xt_r = xt.ap().rearrange("(cc p) t -> p cc t", p=P)
            for qc in range(N_QC):
                for cc in range(N_CC):
                    t_sb = xt_pool.tile([P, CHUNK], bf16, tag=f"xt{cc}_{qc}")
                    # sync (SP) queue: keeps DMA-driving off the Act engine,
                    # whose exp stream must start early. All of qc0 on the
                    # (initially idle) gpsimd queue so head 0 starts fast.
                    eng = nc.gpsimd if qc == 0 else nc.sync
                    eng.dma_start(
                        out=t_sb,
                        in_=xt_r[:, cc, qc * CHUNK:(qc + 1) * CHUNK])
                    xt_t[(cc, qc)] = t_sb

            aot_sb = aot_pool.tile([P, HPC, T], bf16)  # attn-out^T, all heads

            # ---- work items ----
            def qk_block(w_sb, out_sb, qc, dve_sub=False):
                """One 512-wide projection block with RoPE applied."""
                ps = ph1_psum.tile([P, CHUNK], fp32, tag="ph1")
                for cc in range(N_CC):
                    nc.tensor.matmul(
                        ps, lhsT=w_sb[:, cc, :], rhs=xt_t[(cc, qc)],
                        start=(cc == 0), stop=(cc == N_CC - 1))
                # RoPE: rows 0:64 = even dims (e), 64:128 = odd dims (o)
                # re = e*cos - o*sin ; ro = o*cos + e*sin
                v_t = vw_pool.tile([P, CHUNK], bf16, tag="vw")   # [e*cos; o*sin]
                w_t = vw_pool.tile([P, CHUNK], bf16, tag="vw")   # [e*sin; o*cos]
                nc.vector.tensor_mul(v_t, ps, cs_sb[:, qc * CHUNK:(qc + 1) * CHUNK])
                nc.vector.tensor_mul(w_t, ps, sc_sb[:, qc * CHUNK:(qc + 1) * CHUNK])
                sw = swp_pool.tile([P, CHUNK], bf16, tag="swp")
                # sw[0:64] <- o*sin (from rows 64:128); sw[64:128] <- e*sin
                nc.gpsimd.dma_start(out=sw[0:64, :], in_=v_t[64:128, :])
                nc.gpsimd.dma_start(out=sw[64:128, :], in_=w_t[0:64, :])
                sl = slice(qc * CHUNK, (qc + 1) * CHUNK)
                # odd heads run the sub on DVE (idle there: no V drains), so
                # the two RoPE halves finish in parallel and the serial
                # gpsimd chain stops gating attention's diagonal scores
                sub_eng = nc.vector if dve_sub else nc.gpsimd
                sub_eng.tensor_sub(out_sb[0:64, sl], v_t[0:64, :], sw[0:64, :])
                nc.gpsimd.tensor_add(out_sb[64:128, sl], w_t[64:128, :], sw[64:128, :])

            def v_block(w_sb, v_sb, tt0):
                """Four t-tiles of the V projection for one head pair."""
                for tt in range(tt0, tt0 + 4):
                    ps = ph1_psum.tile([P, CHUNK], fp32, tag="ph1")
                    psv = ps[:, 0:2 * D]
                    t0 = (tt % 4) * P
                    for cc in range(N_CC):
                        nc.tensor.matmul(
                            psv, lhsT=xt_t[(cc, tt // 4)][:, t0:t0 + P],
                            rhs=w_sb[:, cc, :],
                            start=(cc == 0), stop=(cc == N_CC - 1))
                    # drain on DVE: keeps V-psum recycling off the Act engine
                    nc.vector.tensor_scalar_add(v_sb[:, tt, :], psv, 0.0)

            v_tiles = {}
            qk_tiles = {}

            def produce_items(h):
                wq_sb, wk_sb = qk_w.pop(h)
                qtr = qk_pool.tile([P, T], bf16, tag="wqtr", name=f"qtr{h}")
                ktr = qk_pool.tile([P, T], bf16, tag="wktr", name=f"ktr{h}")
                qk_tiles[h] = (qtr, ktr)
                items = []
                # odd heads split RoPE sub/add across DVE+gpsimd: DVE has no
                # V-drain duty there, and the parallel halves keep the serial
                # gpsimd chain off attention's critical path
                dve_sub = h % 2 == 1
                for qc in range(N_QC):
                    items.append(lambda qc=qc: qk_block(wq_sb, qtr, qc, dve_sub))
                for qc in range(N_QC):
                    items.append(lambda qc=qc: qk_block(wk_sb, ktr, qc, dve_sub))
                if h % 2 == 0:
                    w_sb = wv_w.pop(h // 2)
                    v_sb = v_pool.tile([P, N_KT, 2 * D], bf16, tag="vsb",
                                       name=f"vsb{h // 2}")
                    v_tiles[h // 2] = v_sb
                    for g in range(4):
                        items.append(lambda g=g, w=w_sb, v=v_sb: v_block(w, v, 4 * g))
                return items

            def attention_qc(h, qc, qtr, ktr, v_sb, v_col):
                """One q-chunk of causal attention.

                Full-width (off-diagonal) k-tiles are processed two at a
                time with a single paired exp instruction, amortizing the
                Act engine's fixed per-instruction overhead so exp keeps
                ahead of the PE's per-tile matmul rate.
                """
                q0 = qc * CHUNK
                ps_o = o_psum.tile([P, CHUNK], fp32, tag="po")
                ps_r = r_psum.tile([P, CHUNK], fp32, tag="pr")
                n_kt = 4 * qc + 4

                def v_r_mm(kt, pt_ap, n0):
                    nc.tensor.matmul(
                        ps_o[:, n0:CHUNK],
                        lhsT=v_sb[:, kt, v_col * D:(v_col + 1) * D],
                        rhs=pt_ap, start=(kt == 0), stop=(kt == n_kt - 1))
                    nc.tensor.matmul(
                        ps_r[:, n0:CHUNK], lhsT=ones_sb, rhs=pt_ap,
                        start=(kt == 0), stop=(kt == n_kt - 1))

                kt = 0
                while kt < n_kt:
                    j = kt - 4 * qc  # >= 0 on diagonal tiles
                    if j < 0 and kt + 1 < 4 * qc + 1:
                        # pair of full-width off-diagonal tiles sharing one
                        # exp instruction (amortizes Act per-instr overhead)
                        ps2 = s_psum.tile([P, 2, CHUNK], fp32, tag="pst")
                        for i in range(2):
                            nc.tensor.matmul(
                                ps2[:, i, :],
                                lhsT=ktr[:, (kt + i) * P:(kt + i + 1) * P],
                                rhs=qtr[:, q0:q0 + CHUNK],
                                start=True, stop=True)
                        pt2 = pt_pool.tile([P, 2, CHUNK], bf16, tag="pt")
                        nc.scalar.activation(pt2, ps2, Exp, scale=SCALE)
                        for i in range(2):
                            v_r_mm(kt + i, pt2[:, i, :], 0)
                        kt += 2
                    else:
                        n0 = max(0, j * P)  # trimmed col start within chunk
                        w = CHUNK - n0
                        ps_s = s_psum.tile([P, 2, CHUNK], fp32, tag="pst")
                        nc.tensor.matmul(
                            ps_s[:, 0, 0:w], lhsT=ktr[:, kt * P:(kt + 1) * P],
                            rhs=qtr[:, q0 + n0:q0 + CHUNK],
                            start=True, stop=True)
                        pt = pt_pool.tile([P, 2, CHUNK], bf16, tag="pt")
                        nc.scalar.activation(pt[:, 0, 0:w], ps_s[:, 0, 0:w],
                                             Exp, scale=SCALE)
                        if j >= 0:
                            # triangle mask on first P cols of trimmed range
                            nc.vector.tensor_mul(pt[:, 0, 0:P], pt[:, 0, 0:P],
                                                 tri_sb)
                        v_r_mm(kt, pt[:, 0, 0:w], n0)
                        kt += 1
                rec = rcp_pool.tile([P, CHUNK], fp32, tag="rcp")
                nc.vector.reciprocal_approx_fast(out=rec, in_=ps_r)
                nc.vector.tensor_mul(aot_sb[:, h, q0:q0 + CHUNK], ps_o, rec)

            def attention_items(h):
                qtr, ktr = qk_tiles.pop(h)
                v_sb = v_tiles[h // 2]
                v_col = h % 2
                return [
                    lambda qc=qc: attention_qc(h, qc, qtr, ktr, v_sb, v_col)
                    for qc in range(N_QC)
                ]

            wo_tiles = {}

            def load_wo(co):
                tiles = []
                for cb in range(HPC):  # contraction chunks == heads
                    w_sb = wo_pool.tile([P, CHUNK], bf16, tag="wo")
                    nc.sync.dma_start(
                        out=w_sb,
                        in_=wo.ap()[cb * P:(cb + 1) * P,
                                    co * CHUNK:(co + 1) * CHUNK])
                    tiles.append(w_sb)
                wo_tiles[co] = tiles

            def ph3_group(co, g):
                """Four t-tiles of the output projection for column group co."""
                tiles = wo_tiles[co]
                for tt in range(4 * g, 4 * g + 4):
                    ps = ph1_psum.tile([P, CHUNK], fp32, tag="ph1")
                    for cb in range(HPC):
                        nc.tensor.matmul(
                            ps, lhsT=aot_sb[:, cb, tt * P:(tt + 1) * P],
                            rhs=tiles[cb],
                            start=(cb == 0), stop=(cb == HPC - 1))
                    yt = y_pool.tile([P, CHUNK], bf16, tag="y")
                    # alternate drains between Act and DVE so neither engine's
                    # in-order stream delays attention work in the head-7
                    # region where phase-3 overlaps attention
                    if tt % 2 == 0:
                        nc.vector.tensor_scalar_add(yt, ps, 0.0)
                    else:
                        nc.scalar.copy(yt, ps)
                    # alternate write-back queues to shorten the drain tail
                    deng = nc.sync if tt % 2 == 0 else nc.scalar
                    deng.dma_start(
                        out=y.ap()[tt * P:(tt + 1) * P,
                                   co * CHUNK:(co + 1) * CHUNK],
                        in_=yt)

            # ---- head steps: produce(h) gate-interleaved with attention(h).
            # attention(h).qc needs only Q-block qc, K-blocks <= qc and
            # V-groups <= qc of its own head, so it lags produce(h) by a
            # couple of items. Act's exp work spreads over the whole step.
            for h in range(HPC):
                if h + 1 < HPC:
                    load_qk_w(h + 1)
                    if (h + 1) % 2 == 0:
                        load_wv((h + 1) // 2)
                if h == HPC - 2:
                    load_wo(0)
                if h == HPC - 1:
                    load_wo(1)
                p = produce_items(h)     # [Q0..Q3, K0..K3, (V0..V3)]
                a = attention_items(h)
                last = h == HPC - 1
                if len(p) == 12:  # even head: Q, K, V blocks
                    for qc in range(N_QC):
                        p[qc]()          # Q block qc
                        p[4 + qc]()      # K block qc
                        p[8 + qc]()      # V group qc
                        a[qc]()
                        if last:
                            ph3_group(0, qc)
                elif not last:    # odd head: Q, K with spacer ordering
                    order = [0, 4, 1, (0,), 5, 2, (1,), 6, 3, (2,), 7, (3,)]
                    for o in order:
                        if isinstance(o, tuple):
                            a[o[0]]()
                        else:
                            p[o]()
                else:
                    # head 7: all projections front-loaded so every RoPE
                    # chain drains before the attention tail; the resulting
                    # Act-engine lag is free because the following phase-3
                    # groups need no Act work
                    for o in [0, 4, 1, (0,), 5, 2, 3, (1,), 6, 7, (2,)]:
                        if isinstance(o, tuple):
                            a[o[0]]()
                        else:
                            p[o]()
                    ph3_group(0, 0)
                    a[3]()
                    for g in range(1, 4):
                        ph3_group(0, g)

            # ---- phase 3: remaining output-projection column groups ----
            for co in range(1, N_QC):
                if co + 1 < N_QC:
                    load_wo(co + 1)
                for g in range(4):
                    ph3_group(co, g)

    nc.finalize()
    return nc


def _host_prep(x, Wq, Wk, Wv, Wo):
    """Build the 8 per-core input maps."""
    perm = np.concatenate([np.arange(0, D, 2), np.arange(1, D, 2)])

    inv_freq = 1.0 / ROPE_THETA ** (np.arange(0, D, 2, dtype=np.float32) / D)
    pos = np.arange(T, dtype=np.float32)
    freqs = np.einsum("i,j->ij", pos, inv_freq)  # [T, 64]
    cosT = np.cos(freqs).T.astype(np.float32)
    sinT = np.sin(freqs).T.astype(np.float32)
    cs = np.concatenate([cosT, sinT], 0).astype(BF16)
    sc = np.concatenate([sinT, cosT], 0).astype(BF16)
    tri = (np.arange(P)[:, None] <= np.arange(P)[None, :]).astype(BF16)

    halves = []
    for g in range(2):
        heads = range(g * HPC, (g + 1) * HPC)
        wq_g = np.stack([Wq[h * D:(h + 1) * D, :][perm, :].T.astype(BF16)
                         for h in heads])                       # [8, C, D]
        wk_g = np.stack([Wk[h * D:(h + 1) * D, :][perm, :].T.astype(BF16)
                         for h in heads])
        wv_g = np.stack([Wv[(g * HPC + 2 * p) * D:(g * HPC + 2 * p + 2) * D, :]
                         .T.astype(BF16) for p in range(HPC // 2)])  # [4, C, 2D]
        wo_g = Wo[:, g * HPC * D:(g + 1) * HPC * D].T.astype(BF16)   # [1024, C]
        halves.append(dict(wq=wq_g, wk=wk_g, wv=wv_g, wo=wo_g))

    in_maps = []
    for core in range(N_CORES):
        b, g = core // 2, core % 2
        m = dict(halves[g])
        m["xt"] = np.ascontiguousarray(x[b].T).astype(BF16)
        m["cs"] = cs
        m["sc"] = sc
        m["tri"] = tri
        in_maps.append(m)
    return in_maps


def kernel(x, Wq, Wk, Wv, Wo, bo):
    x = np.asarray(x, dtype=np.float32)
    Wq = np.asarray(Wq, dtype=np.float32)
    Wk = np.asarray(Wk, dtype=np.float32)
    Wv = np.asarray(Wv, dtype=np.float32)
    Wo = np.asarray(Wo, dtype=np.float32)
    bo = np.asarray(bo, dtype=np.float32)

    if "nc" not in _CACHED:
        _CACHED["nc"] = build_kernel()
    nc = _CACHED["nc"]

    in_maps = _host_prep(x, Wq, Wk, Wv, Wo)
    res = run_bass_kernel_spmd(nc, in_maps, core_ids=list(range(N_CORES)))

    out = np.empty((B, T, C), np.float32)
    for b in range(B):
        out[b] = (res.results[2 * b]["y"].astype(np.float32)
                  + res.results[2 * b + 1]["y"].astype(np.float32) + bo)
    return out



# revision 4
# speedup vs baseline: 1.0361x; 1.0361x over previous
"""Trainium2 Bass kernel: multi-head attention with RoPE (causal), 8-core SPMD.

Sharding: 8 cores = 4 batches x 2 head-halves (tensor parallel over heads,
data parallel over batch). Each core computes, for its batch b and its 8
heads: QKV projections, RoPE, causal attention, and a partial output
projection. Host sums the two head-half partials per batch and adds the bias.

All TensorEngine math in fp16 with fp32 PSUM accumulation.

v2 changes vs the bf16 baseline:
- softmax denominator no longer uses a per-k-tile ones-matmul on the PE
  (which cost ~58us of PE stream); exp tiles are accumulated into a
  per-(h,qc) pt_sum on DVE/Pool and a single 512-col ones-matmul per
  (h,qc) produces the row sums.
- RoPE has no SBUF-SBUF swap DMAs: with cs=[cos;cos], sc=[sin;sin] the
  cross-partition combine is done directly by DVE tensor ops whose PSUM
  operand may use a different base partition than the SBUF operand
  (verifier only forbids misaligned bases when BOTH inputs are in SBUF).
- fp16 instead of bf16 (same PE rate, 2x DVE mode, more mantissa).
- head-0 weight loads are chunked so the first matmul starts ~2us in.
- V-projection PSUM drains moved to the Act engine (same act table as
  Exp, so no table-switch cost).

Schedule: attention(h) is interleaved with produce(h) at qc/block
granularity; the per-(h,qc) finalize (ones-matmul + reciprocal + norm)
is delayed a couple of items so the PE never waits on the DVE add chain.

DMA queues: weights + consts + xt qc1-3 + y write-back on sync; cs/sc and
half the y write-back on scalar; xt qc0 on gpsimd.
"""

import sys

for _p in ("/opt/trn_rl_repo",):
    if _p not in sys.path:
        sys.path.insert(0, _p)

import numpy as np

import concourse.bass as bass
import concourse.bacc as bacc
import concourse.mybir as mybir
import concourse.tile as tile
from concourse.bass_utils import run_bass_kernel_spmd

F16 = np.float16

B, T, C = 4, 2048, 2048
H = 16
D = C // H  # 128
ROPE_THETA = 1000000.0
N_CORES = 8
HPC = H // 2          # heads per core (8)
P = 128               # partitions
CHUNK = 512           # moving free dim per matmul
N_CC = C // P         # 16 contraction chunks
N_QC = T // CHUNK     # 4 q-chunks
N_KT = T // P         # 16 k-tiles
SCALE = 1.0 / float(np.sqrt(D))
NWC = 4               # weight-load chunks per projection matrix

_CACHED = {}


def build_kernel():
    """Build the SPMD Bass program (identical on all 8 cores)."""
    fp32 = mybir.dt.float32
    fp16 = mybir.dt.float16

    nc = bacc.Bacc("TRN2", target_bir_lowering=False, debug=False,
                   num_devices=N_CORES)

    # Per-core DRAM inputs (fp16 unless noted)
    xt = nc.dram_tensor("xt", [C, T], fp16, kind="ExternalInput")          # x[b].T
    wq = nc.dram_tensor("wq", [HPC, C, D], fp16, kind="ExternalInput")     # Wq_h.T (perm'd)
    wk = nc.dram_tensor("wk", [HPC, C, D], fp16, kind="ExternalInput")
    wv = nc.dram_tensor("wv", [HPC // 2, C, 2 * D], fp16, kind="ExternalInput")  # head pairs
    wo = nc.dram_tensor("wo", [HPC * D, C], fp16, kind="ExternalInput")    # WoT slice
    cs = nc.dram_tensor("cs", [P, T], fp16, kind="ExternalInput")          # [cosT;cosT]
    sc = nc.dram_tensor("sc", [P, T], fp16, kind="ExternalInput")          # [sinT;sinT]
    tri = nc.dram_tensor("tri", [P, P], fp16, kind="ExternalInput")        # k<=q mask
    y = nc.dram_tensor("y", [T, C], fp16, kind="ExternalOutput")

    Exp = mybir.ActivationFunctionType.Exp

    with tile.TileContext(nc) as tc:
        with (
            tc.tile_pool(name="const", bufs=1) as const_pool,
            tc.tile_pool(name="xtp", bufs=1) as xt_pool,
            tc.tile_pool(name="wqk", bufs=4) as wqk_pool,
            tc.tile_pool(name="wvp", bufs=2) as wv_pool,
            tc.tile_pool(name="bt", bufs=4) as b_pool,
            tc.tile_pool(name="qktr", bufs=2) as qk_pool,
            tc.tile_pool(name="vsb", bufs=2) as v_pool,
            tc.tile_pool(name="pt", bufs=3) as pt_pool,
            tc.tile_pool(name="pts", bufs=2) as pts_pool,
            tc.tile_pool(name="tmp", bufs=2) as tmp_pool,
            tc.tile_pool(name="rcp", bufs=2) as rcp_pool,
            tc.tile_pool(name="aot", bufs=1) as aot_pool,
            tc.tile_pool(name="wop", bufs=16) as wo_pool,
            tc.tile_pool(name="yst", bufs=3) as y_pool,
            tc.tile_pool(name="ph1", bufs=2, space="PSUM") as ph1_psum,
            tc.tile_pool(name="pst", bufs=2, space="PSUM") as s_psum,
            tc.tile_pool(name="po", bufs=1, space="PSUM") as o_psum,
            tc.tile_pool(name="pr", bufs=1, space="PSUM") as r_psum,
        ):
            # ---- weight loads: chunked on the sync queue ----
            qk_w = {}

            def load_qk_w(h):
                wq_r = wq.ap()[h].rearrange("(cc p) d -> p cc d", p=P)
                wk_r = wk.ap()[h].rearrange("(cc p) d -> p cc d", p=P)
                tiles = []
                for src in (wq_r, wk_r):
                    for ch in range(NWC):
                        t_sb = wqk_pool.tile([P, N_CC // NWC, D], fp16,
                                             tag=f"wqk{ch}")
                        nc.sync.dma_start(
                            out=t_sb,
                            in_=src[:, ch * (N_CC // NWC):(ch + 1) * (N_CC // NWC), :])
                        tiles.append(t_sb)
                qk_w[h] = tiles  # [wq c0..c3, wk c0..c3]

            wv_w = {}

            def load_wv(pair):
                wv_r = wv.ap()[pair].rearrange("(cc p) d -> p cc d", p=P)
                tiles = []
                for ch in range(NWC):
                    t_sb = wv_pool.tile([P, N_CC // NWC, 2 * D], fp16,
                                        tag=f"wv{ch}")
                    nc.sync.dma_start(
                        out=t_sb,
                        in_=wv_r[:, ch * (N_CC // NWC):(ch + 1) * (N_CC // NWC), :])
                    tiles.append(t_sb)
                wv_w[pair] = tiles

            load_qk_w(0)

            # ---- constants (scalar queue, parallel with sync weights) ----
            cs_sb = const_pool.tile([P, T], fp16)
            sc_sb = const_pool.tile([P, T], fp16)
            tri_sb = const_pool.tile([P, P], fp16)
            ones_sb = const_pool.tile([P, P], fp16)
            nc.scalar.dma_start(out=cs_sb, in_=cs.ap())
            nc.scalar.dma_start(out=sc_sb, in_=sc.ap())
            nc.scalar.dma_start(out=tri_sb, in_=tri.ap())
            load_wv(0)
            nc.vector.memset(ones_sb, 1.0)

            # ---- x^T load: 64 chunk tiles ----
            xt_t = {}
            xt_r = xt.ap().rearrange("(cc p) t -> p cc t", p=P)
            for qc in range(N_QC):
                for cc in range(N_CC):
                    t_sb = xt_pool.tile([P, CHUNK], fp16, tag=f"xt{cc}_{qc}")
                    # qc0 on the gpsimd queue so head 0 starts fast; the
                    # rest stream on sync behind the head-0 weights.
                    eng = nc.gpsimd if qc == 0 else nc.sync
                    eng.dma_start(
                        out=t_sb,
                        in_=xt_r[:, cc, qc * CHUNK:(qc + 1) * CHUNK])
                    xt_t[(cc, qc)] = t_sb

            aot_sb = aot_pool.tile([P, HPC, T], fp16)  # attn-out^T, all heads

            # ---- work items ----
            def qk_block(w_chunks, out_sb, qc):
                """One 512-wide projection block with RoPE applied.

                RoPE without swap copies: with cs=[cos;cos], sc=[sin;sin],
                  b  = ps * sc          (SBUF)   = [e*sin; o*sin]
                  ps = ps * cs          (inplace)= [e*cos; o*cos]
                  re[0:64]   = ps[0:64]   - b[64:128]
                  ro[64:128] = ps[64:128] + b[0:64]
                The cross-partition ops are legal because one operand is in
                PSUM (misaligned bases are only rejected for SBUF+SBUF).
                """
                ps = ph1_psum.tile([P, CHUNK], fp32, tag="ph1")
                for cc in range(N_CC):
                    nc.tensor.matmul(
                        ps, lhsT=w_chunks[cc // NWC][:, cc % NWC, :],
                        rhs=xt_t[(cc, qc)],
                        start=(cc == 0), stop=(cc == N_CC - 1))
                sl = slice(qc * CHUNK, (qc + 1) * CHUNK)
                b_t = b_pool.tile([P, CHUNK], fp16, tag="bt")
                nc.vector.tensor_mul(b_t, ps, sc_sb[:, sl])
                nc.vector.tensor_mul(ps, ps, cs_sb[:, sl])
                nc.vector.tensor_sub(out_sb[0:64, sl], ps[0:64, :],
                                     b_t[64:128, :])
                nc.vector.tensor_add(out_sb[64:128, sl], ps[64:128, :],
                                     b_t[0:64, :])

            def v_block(w_chunks, v_sb, tt0):
                """Four t-tiles of the V projection for one head pair."""
                for tt in range(tt0, tt0 + 4):
                    ps = ph1_psum.tile([P, CHUNK], fp32, tag="ph1")
                    psv = ps[:, 0:2 * D]
                    t0 = (tt % 4) * P
                    for cc in range(N_CC):
                        nc.tensor.matmul(
                            psv, lhsT=xt_t[(cc, tt // 4)][:, t0:t0 + P],
                            rhs=w_chunks[cc // NWC][:, cc % NWC, :],
                            start=(cc == 0), stop=(cc == N_CC - 1))
                    # drain on Act: Exp/Copy share an activation table
                    nc.scalar.copy(v_sb[:, tt, :], psv)

            v_tiles = {}
            qk_tiles = {}

            def produce_items(h):
                w_tiles = qk_w.pop(h)
                qtr = qk_pool.tile([P, T], fp16, tag="wqtr", name=f"qtr{h}")
                ktr = qk_pool.tile([P, T], fp16, tag="wktr", name=f"ktr{h}")
                qk_tiles[h] = (qtr, ktr)
                items = []
                for qc in range(N_QC):
                    items.append(lambda qc=qc: qk_block(w_tiles[:NWC], qtr, qc))
                for qc in range(N_QC):
                    items.append(lambda qc=qc: qk_block(w_tiles[NWC:], ktr, qc))
                if h % 2 == 0:
                    w_v = wv_w.pop(h // 2)
                    v_sb = v_pool.tile([P, N_KT, 2 * D], fp16, tag="vsb",
                                       name=f"vsb{h // 2}")
                    v_tiles[h // 2] = v_sb
                    for g in range(4):
                        items.append(lambda g=g, w=w_v, v=v_sb: v_block(w, v, 4 * g))
                return items

            fin_info = {}

            def attention_qc(h, qc, qtr, ktr, v_sb, v_col):
                """One q-chunk of causal attention (scores + exp + PV).

                The softmax denominator is NOT computed with per-k-tile
                ones-matmuls; instead exp tiles accumulate into pt_sum
                (pair-internal sums on Pool, += on DVE) and finalize_qc
                runs one small ones-matmul on pt_sum.
                """
                q0 = qc * CHUNK
                ps_o = o_psum.tile([P, CHUNK], fp32, tag="po")
                pt_sum = pts_pool.tile([P, CHUNK], fp16, tag="pts")
                n_kt = 4 * qc + 4
                first = [True]

                def acc(dst_sl, src_ap):
                    if first[0]:
                        nc.vector.tensor_scalar_add(pt_sum, src_ap, 0.0)
                        first[0] = False
                    else:
                        nc.vector.tensor_add(pt_sum[:, dst_sl],
                                             pt_sum[:, dst_sl], src_ap)

                def v_mm(kt, pt_ap, n0):
                    nc.tensor.matmul(
                        ps_o[:, n0:CHUNK],
                        lhsT=v_sb[:, kt, v_col * D:(v_col + 1) * D],
                        rhs=pt_ap, start=(kt == 0), stop=(kt == n_kt - 1))

                kt = 0
                while kt < n_kt:
                    j = kt - 4 * qc  # >= 0 on diagonal tiles
                    if j < 0 and kt + 1 < 4 * qc + 1:
                        # pair of full-width off-diagonal tiles sharing one
                        # exp instruction (amortizes Act per-instr overhead)
                        ps2 = s_psum.tile([P, 2, CHUNK], fp32, tag="pst")
                        for i in range(2):
                            nc.tensor.matmul(
                                ps2[:, i, :],
                                lhsT=ktr[:, (kt + i) * P:(kt + i + 1) * P],
                                rhs=qtr[:, q0:q0 + CHUNK],
                                start=True, stop=True)
                        pt2 = pt_pool.tile([P, 2, CHUNK], fp16, tag="pt")
                        nc.scalar.activation(pt2, ps2, Exp, scale=SCALE)
                        # pair-internal sum on Pool, += on DVE
                        tmp = tmp_pool.tile([P, CHUNK], fp16, tag="tmp")
                        nc.gpsimd.tensor_add(tmp, pt2[:, 0, :], pt2[:, 1, :])
                        acc(slice(0, CHUNK), tmp)
                        for i in range(2):
                            v_mm(kt + i, pt2[:, i, :], 0)
                        kt += 2
                    else:
                        n0 = max(0, j * P)  # trimmed col start within chunk
                        w = CHUNK - n0
                        ps_s = s_psum.tile([P, 2, CHUNK], fp32, tag="pst")
                        nc.tensor.matmul(
                            ps_s[:, 0, 0:w], lhsT=ktr[:, kt * P:(kt + 1) * P],
                            rhs=qtr[:, q0 + n0:q0 + CHUNK],
                            start=True, stop=True)
                        pt = pt_pool.tile([P, 2, CHUNK], fp16, tag="pt")
                        nc.scalar.activation(pt[:, 0, 0:w], ps_s[:, 0, 0:w],
                                             Exp, scale=SCALE)
                        if j >= 0:
                            # triangle mask on first P cols of trimmed range
                            nc.vector.tensor_mul(pt[:, 0, 0:P], pt[:, 0, 0:P],
                                                 tri_sb)
                        acc(slice(n0, CHUNK), pt[:, 0, 0:w])
                        v_mm(kt, pt[:, 0, 0:w], n0)
                        kt += 1
                fin_info[(h, qc)] = (ps_o, pt_sum)

            def finalize_qc(h, qc):
                """Denominator ones-matmul + reciprocal + normalization."""
                ps_o, pt_sum = fin_info.pop((h, qc))
                q0 = qc * CHUNK
                ps_r = r_psum.tile([P, CHUNK], fp32, tag="pr")
                nc.tensor.matmul(ps_r, lhsT=ones_sb, rhs=pt_sum,
                                 start=True, stop=True)
                rec = rcp_pool.tile([P, CHUNK], fp32, tag="rcp")
                nc.vector.reciprocal_approx_fast(out=rec, in_=ps_r)
                nc.vector.tensor_mul(aot_sb[:, h, q0:q0 + CHUNK], ps_o, rec)

            def attention_items(h):
                qtr, ktr = qk_tiles.pop(h)
                v_sb = v_tiles[h // 2]
                v_col = h % 2
                return [
                    lambda qc=qc: attention_qc(h, qc, qtr, ktr, v_sb, v_col)
                    for qc in range(N_QC)
                ]

            wo_tiles = {}

            def load_wo(co):
                tiles = []
                for cb in range(HPC):  # contraction chunks == heads
                    w_sb = wo_pool.tile([P, CHUNK], fp16, tag="wo")
                    nc.sync.dma_start(
                        out=w_sb,
                        in_=wo.ap()[cb * P:(cb + 1) * P,
                                    co * CHUNK:(co + 1) * CHUNK])
                    tiles.append(w_sb)
                wo_tiles[co] = tiles

            def ph3_group(co, g, last_group=False):
                """Four t-tiles of the output projection for column group co."""
                tiles = wo_tiles[co]
                for tt in range(4 * g, 4 * g + 4):
                    ps = ph1_psum.tile([P, CHUNK], fp32, tag="ph1")
                    for cb in range(HPC):
                        nc.tensor.matmul(
                            ps, lhsT=aot_sb[:, cb, tt * P:(tt + 1) * P],
                            rhs=tiles[cb],
                            start=(cb == 0), stop=(cb == HPC - 1))
                    yt = y_pool.tile([P, CHUNK], fp16, tag="y")
                    # alternate drains between Act and DVE so neither
                    # engine's in-order stream delays interleaved attention
                    if tt % 2 == 0:
                        nc.vector.tensor_scalar_add(yt, ps, 0.0)
                    else:
                        nc.scalar.copy(yt, ps)
                    if last_group:
                        # spread the final write-backs over three queues so
                        # the drain tail is parallel
                        deng = (nc.sync, nc.scalar, nc.gpsimd, nc.sync)[tt % 4]
                    else:
                        deng = nc.sync if tt % 2 == 0 else nc.scalar
                    deng.dma_start(
                        out=y.ap()[tt * P:(tt + 1) * P,
                                   co * CHUNK:(co + 1) * CHUNK],
                        in_=yt)

            # ---- head steps: produce(h) gate-interleaved with attention(h).
            # attention(h).qc needs only Q-block qc, K-blocks <= qc and
            # V-groups <= qc of its own head, so it lags produce(h) by a
            # couple of items. finalize(h,qc) lags attention(h,qc) by a
            # produce item so the PE's ones-matmul never waits on the DVE
            # accumulation chain.
            pend = []  # delayed finalize carried into the next head

            def flush_pend():
                while pend:
                    pend.pop(0)()

            for h in range(HPC):
                if h + 1 < HPC:
                    load_qk_w(h + 1)
                    if (h + 1) % 2 == 0:
                        load_wv((h + 1) // 2)
                if h == HPC - 2:
                    load_wo(0)
                if h == HPC - 1:
                    load_wo(1)
                p = produce_items(h)     # [Q0..Q3, K0..K3, (V0..V3)]
                a = attention_items(h)
                f = [lambda qc=qc, h=h: finalize_qc(h, qc)
                     for qc in range(N_QC)]
                last = h == HPC - 1
                if len(p) == 12:  # even head: Q, K, V blocks
                    for qc in range(N_QC):
                        p[qc]()          # Q block qc
                        if qc == 0:
                            flush_pend()
                        p[4 + qc]()      # K block qc
                        if qc >= 1:
                            f[qc - 1]()
                        p[8 + qc]()      # V group qc
                        a[qc]()
                    pend.append(f[3])
                elif not last:    # odd head: Q, K with spacer ordering
                    seq = [p[0], flush_pend, p[4], p[1], a[0], p[5],
                           f[0], p[2], a[1], p[6], f[1], p[3], a[2],
                           p[7], f[2], a[3]]
                    for s in seq:
                        s()
                    pend.append(f[3])
                else:
                    # head 7: projections front-loaded; phase-3 groups give
                    # the Act/DVE chains slack to drain the attention tail
                    seq = [p[0], flush_pend, p[4], p[1], a[0], p[5],
                           f[0], p[2], a[1], p[6], p[3], a[2], f[1],
                           p[7], a[3], f[2]]
                    for s in seq:
                        s()
                    ph3_group(0, 0)
                    f[3]()
                    for g in range(1, 4):
                        ph3_group(0, g)

            # ---- phase 3: remaining output-projection column groups ----
            for co in range(1, N_QC):
                if co + 1 < N_QC:
                    load_wo(co + 1)
                for g in range(4):
                    ph3_group(co, g, last_group=(co == N_QC - 1 and g == 3))

    nc.finalize()
    return nc


def _host_prep(x, Wq, Wk, Wv, Wo):
    """Build the 8 per-core input maps."""
    perm = np.concatenate([np.arange(0, D, 2), np.arange(1, D, 2)])

    inv_freq = 1.0 / ROPE_THETA ** (np.arange(0, D, 2, dtype=np.float32) / D)
    pos = np.arange(T, dtype=np.float32)
    freqs = np.einsum("i,j->ij", pos, inv_freq)  # [T, 64]
    cosT = np.cos(freqs).T.astype(np.float32)
    sinT = np.sin(freqs).T.astype(np.float32)
    cs = np.concatenate([cosT, cosT], 0).astype(F16)
    sc = np.concatenate([sinT, sinT], 0).astype(F16)
    tri = (np.arange(P)[:, None] <= np.arange(P)[None, :]).astype(F16)

    halves = []
    for g in range(2):
        heads = range(g * HPC, (g + 1) * HPC)
        wq_g = np.stack([Wq[h * D:(h + 1) * D, :][perm, :].T.astype(F16)
                         for h in heads])                       # [8, C, D]
        wk_g = np.stack([Wk[h * D:(h + 1) * D, :][perm, :].T.astype(F16)
                         for h in heads])
        wv_g = np.stack([Wv[(g * HPC + 2 * p) * D:(g * HPC + 2 * p + 2) * D, :]
                         .T.astype(F16) for p in range(HPC // 2)])  # [4, C, 2D]
        wo_g = Wo[:, g * HPC * D:(g + 1) * HPC * D].T.astype(F16)   # [1024, C]
        halves.append(dict(wq=wq_g, wk=wk_g, wv=wv_g, wo=wo_g))

    in_maps = []
    for core in range(N_CORES):
        b, g = core // 2, core % 2
        m = dict(halves[g])
        m["xt"] = np.ascontiguousarray(x[b].T).astype(F16)
        m["cs"] = cs
        m["sc"] = sc
        m["tri"] = tri
        in_maps.append(m)
    return in_maps


def kernel(x, Wq, Wk, Wv, Wo, bo):
    x = np.asarray(x, dtype=np.float32)
    Wq = np.asarray(Wq, dtype=np.float32)
    Wk = np.asarray(Wk, dtype=np.float32)
    Wv = np.asarray(Wv, dtype=np.float32)
    Wo = np.asarray(Wo, dtype=np.float32)
    bo = np.asarray(bo, dtype=np.float32)

    if "nc" not in _CACHED:
        _CACHED["nc"] = build_kernel()
    nc = _CACHED["nc"]

    in_maps = _host_prep(x, Wq, Wk, Wv, Wo)
    res = run_bass_kernel_spmd(nc, in_maps, core_ids=list(range(N_CORES)))

    out = np.empty((B, T, C), np.float32)
    for b in range(B):
        out[b] = (res.results[2 * b]["y"].astype(np.float32)
                  + res.results[2 * b + 1]["y"].astype(np.float32) + bo)
    return out


# revision 12
# speedup vs baseline: 1.0434x; 1.0071x over previous
"""Trainium2 Bass kernel: multi-head attention with RoPE (causal), 8-core SPMD.

Sharding: 8 cores = 4 batches x 2 head-halves (tensor parallel over heads,
data parallel over batch). Each core computes, for its batch b and its 8
heads: QKV projections, RoPE, causal attention, and a partial output
projection. Host sums the two head-half partials per batch and adds the bias.

All TensorEngine math in fp16 with fp32 PSUM accumulation.

v2 changes vs the bf16 baseline:
- softmax denominator no longer uses a per-k-tile ones-matmul on the PE
  (which cost ~58us of PE stream); exp tiles are accumulated into a
  per-(h,qc) pt_sum on DVE/Pool and a single 512-col ones-matmul per
  (h,qc) produces the row sums.
- RoPE has no SBUF-SBUF swap DMAs: with cs=[cos;cos], sc=[sin;sin] the
  cross-partition combine is done directly by DVE tensor ops whose PSUM
  operand may use a different base partition than the SBUF operand
  (verifier only forbids misaligned bases when BOTH inputs are in SBUF).
- fp16 instead of bf16 (same PE rate, 2x DVE mode, more mantissa).
- head-0 weight loads are chunked so the first matmul starts ~2us in.
- V-projection PSUM drains moved to the Act engine (same act table as
  Exp, so no table-switch cost).

Schedule: attention(h) is interleaved with produce(h) at qc/block
granularity; the per-(h,qc) finalize (ones-matmul + reciprocal + norm)
is delayed a couple of items so the PE never waits on the DVE add chain.

DMA queues: weights + consts + xt qc1-3 + y write-back on sync; cs/sc and
half the y write-back on scalar; xt qc0 on gpsimd.
"""

import sys

for _p in ("/opt/trn_rl_repo",):
    if _p not in sys.path:
        sys.path.insert(0, _p)

import numpy as np

import concourse.bass as bass
import concourse.bacc as bacc
import concourse.mybir as mybir
import concourse.tile as tile
from concourse.bass_utils import run_bass_kernel_spmd

F16 = np.float16

B, T, C = 4, 2048, 2048
H = 16
D = C // H  # 128
ROPE_THETA = 1000000.0
N_CORES = 8
HPC = H // 2          # heads per core (8)
P = 128               # partitions
CHUNK = 512           # moving free dim per matmul
N_CC = C // P         # 16 contraction chunks
N_QC = T // CHUNK     # 4 q-chunks
N_KT = T // P         # 16 k-tiles
SCALE = 1.0 / float(np.sqrt(D))
NWC = 4               # weight-load chunks per projection matrix

_CACHED = {}


def build_kernel():
    """Build the SPMD Bass program (identical on all 8 cores)."""
    fp32 = mybir.dt.float32
    fp16 = mybir.dt.float16

    nc = bacc.Bacc("TRN2", target_bir_lowering=False, debug=False,
                   num_devices=N_CORES)

    # Per-core DRAM inputs (fp16 unless noted)
    xt = nc.dram_tensor("xt", [C, T], fp16, kind="ExternalInput")          # x[b].T
    wq = nc.dram_tensor("wq", [HPC, C, D], fp16, kind="ExternalInput")     # Wq_h.T (perm'd)
    wk = nc.dram_tensor("wk", [HPC, C, D], fp16, kind="ExternalInput")
    wv = nc.dram_tensor("wv", [HPC // 2, C, 2 * D], fp16, kind="ExternalInput")  # head pairs
    wo = nc.dram_tensor("wo", [HPC * D, C], fp16, kind="ExternalInput")    # WoT slice
    cs = nc.dram_tensor("cs", [P, T], fp16, kind="ExternalInput")          # [cosT;cosT]
    sc = nc.dram_tensor("sc", [P, T], fp16, kind="ExternalInput")          # [sinT;sinT]
    tri = nc.dram_tensor("tri", [P, P], fp16, kind="ExternalInput")        # k<=q mask
    y = nc.dram_tensor("y", [T, C], fp16, kind="ExternalOutput")

    Exp = mybir.ActivationFunctionType.Exp

    with tile.TileContext(nc) as tc:
        with (
            tc.tile_pool(name="const", bufs=1) as const_pool,
            tc.tile_pool(name="xtp", bufs=1) as xt_pool,
            tc.tile_pool(name="wqk", bufs=4) as wqk_pool,
            tc.tile_pool(name="wvp", bufs=2) as wv_pool,
            tc.tile_pool(name="bt", bufs=2) as b_pool,
            tc.tile_pool(name="qktr", bufs=2) as qk_pool,
            tc.tile_pool(name="vsb", bufs=2) as v_pool,
            tc.tile_pool(name="pt", bufs=4) as pt_pool,
            tc.tile_pool(name="pts", bufs=2) as pts_pool,
            tc.tile_pool(name="tmp", bufs=2) as tmp_pool,
            tc.tile_pool(name="rcp", bufs=2) as rcp_pool,
            tc.tile_pool(name="aot", bufs=1) as aot_pool,
            tc.tile_pool(name="wop", bufs=16) as wo_pool,
            tc.tile_pool(name="yst", bufs=2) as y_pool,
            tc.tile_pool(name="ph1", bufs=2, space="PSUM") as ph1_psum,
            tc.tile_pool(name="pst", bufs=2, space="PSUM") as s_psum,
            tc.tile_pool(name="po", bufs=1, space="PSUM") as o_psum,
            tc.tile_pool(name="pr", bufs=1, space="PSUM") as r_psum,
        ):
            # ---- weight loads: chunked on the sync queue ----
            qk_w = {}

            def load_qk_w(h):
                wq_r = wq.ap()[h].rearrange("(cc p) d -> p cc d", p=P)
                wk_r = wk.ap()[h].rearrange("(cc p) d -> p cc d", p=P)
                tiles = []
                for src in (wq_r, wk_r):
                    for ch in range(NWC):
                        t_sb = wqk_pool.tile([P, N_CC // NWC, D], fp16,
                                             tag=f"wqk{ch}")
                        nc.sync.dma_start(
                            out=t_sb,
                            in_=src[:, ch * (N_CC // NWC):(ch + 1) * (N_CC // NWC), :])
                        tiles.append(t_sb)
                qk_w[h] = tiles  # [wq c0..c3, wk c0..c3]

            wv_w = {}

            def load_wv(pair):
                wv_r = wv.ap()[pair].rearrange("(cc p) d -> p cc d", p=P)
                tiles = []
                for ch in range(NWC):
                    t_sb = wv_pool.tile([P, N_CC // NWC, 2 * D], fp16,
                                        tag=f"wv{ch}")
                    nc.sync.dma_start(
                        out=t_sb,
                        in_=wv_r[:, ch * (N_CC // NWC):(ch + 1) * (N_CC // NWC), :])
                    tiles.append(t_sb)
                wv_w[pair] = tiles

            load_qk_w(0)

            # ---- constants (scalar queue, parallel with sync weights) ----
            cs_sb = const_pool.tile([P, T], fp16)
            sc_sb = const_pool.tile([P, T], fp16)
            tri2_sb = const_pool.tile([P, 2, P], fp16)
            ones_sb = const_pool.tile([P, P], fp16)
            nc.scalar.dma_start(out=cs_sb, in_=cs.ap())
            nc.scalar.dma_start(out=sc_sb, in_=sc.ap())
            nc.scalar.dma_start(out=tri2_sb[:, 0, :], in_=tri.ap())
            nc.scalar.dma_start(out=tri2_sb[:, 1, :], in_=tri.ap())
            load_wv(0)
            nc.vector.memset(ones_sb, 1.0)

            # ---- x^T load: 64 chunk tiles ----
            xt_t = {}
            xt_r = xt.ap().rearrange("(cc p) t -> p cc t", p=P)
            for qc in range(N_QC):
                for cc in range(N_CC):
                    t_sb = xt_pool.tile([P, CHUNK], fp16, tag=f"xt{cc}_{qc}")
                    # qc0 on the gpsimd queue so head 0 starts fast; the
                    # rest stream on sync behind the head-0 weights.
                    eng = nc.gpsimd if qc == 0 else nc.sync
                    eng.dma_start(
                        out=t_sb,
                        in_=xt_r[:, cc, qc * CHUNK:(qc + 1) * CHUNK])
                    xt_t[(cc, qc)] = t_sb

            aot_sb = aot_pool.tile([P, HPC, T], fp16)  # attn-out^T, all heads

            # ---- work items ----
            def qk_block(w_chunks, out_sb, qc):
                """One 512-wide projection block with RoPE applied.

                RoPE without swap DMAs: the Act engine drains the PSUM
                block twice -- straight (qs=[e;o]) and partition-swapped
                (qw=[o;e]) -- so every DVE op below is an aligned all-SBUF
                fp16 op that runs in the 2x DVE mode:
                  qs *= cs ([cos;cos])  -> [e*cos; o*cos]
                  qw *= sc ([sin;sin])  -> [o*sin; e*sin]
                  re[0:64]   = qs[0:64]   - qw[0:64]
                  ro[64:128] = qs[64:128] + qw[64:128]
                This also releases the PSUM tile after the (fast) Act
                drains instead of at the end of the DVE chain.
                """
                ps = ph1_psum.tile([P, CHUNK], fp32, tag="ph1")
                for cc in range(N_CC):
                    nc.tensor.matmul(
                        ps, lhsT=w_chunks[cc // NWC][:, cc % NWC, :],
                        rhs=xt_t[(cc, qc)],
                        start=(cc == 0), stop=(cc == N_CC - 1))
                sl = slice(qc * CHUNK, (qc + 1) * CHUNK)
                qs = b_pool.tile([P, CHUNK], fp16, tag="qs")
                qw = b_pool.tile([P, CHUNK], fp16, tag="qw")
                nc.scalar.copy(qs, ps)
                nc.scalar.copy(qw[0:64, :], ps[64:128, :])
                nc.scalar.copy(qw[64:128, :], ps[0:64, :])
                nc.vector.tensor_mul(qs, qs, cs_sb[:, sl])
                nc.vector.tensor_mul(qw, qw, sc_sb[:, sl])
                nc.vector.tensor_sub(out_sb[0:64, sl], qs[0:64, :],
                                     qw[0:64, :])
                nc.vector.tensor_add(out_sb[64:128, sl], qs[64:128, :],
                                     qw[64:128, :])

            def v_block(w_chunks, v_sb, tt0):
                """Four t-tiles of the V projection for one head pair."""
                for tt in range(tt0, tt0 + 4):
                    ps = ph1_psum.tile([P, CHUNK], fp32, tag="ph1")
                    psv = ps[:, 0:2 * D]
                    t0 = (tt % 4) * P
                    for cc in range(N_CC):
                        nc.tensor.matmul(
                            psv, lhsT=xt_t[(cc, tt // 4)][:, t0:t0 + P],
                            rhs=w_chunks[cc // NWC][:, cc % NWC, :],
                            start=(cc == 0), stop=(cc == N_CC - 1))
                    # drain on Act: Exp/Copy share an activation table
                    nc.scalar.copy(v_sb[:, tt, :], psv)

            v_tiles = {}
            qk_tiles = {}

            def produce_items(h):
                w_tiles = qk_w.pop(h)
                qtr = qk_pool.tile([P, T], fp16, tag="wqtr", name=f"qtr{h}")
                ktr = qk_pool.tile([P, T], fp16, tag="wktr", name=f"ktr{h}")
                qk_tiles[h] = (qtr, ktr)
                items = []
                for qc in range(N_QC):
                    items.append(lambda qc=qc: qk_block(w_tiles[:NWC], qtr, qc))
                for qc in range(N_QC):
                    items.append(lambda qc=qc: qk_block(w_tiles[NWC:], ktr, qc))
                if h % 2 == 0:
                    w_v = wv_w.pop(h // 2)
                    v_sb = v_pool.tile([P, N_KT, 2 * D], fp16, tag="vsb",
                                       name=f"vsb{h // 2}")
                    v_tiles[h // 2] = v_sb
                    for g in range(4):
                        items.append(lambda g=g, w=w_v, v=v_sb: v_block(w, v, 4 * g))
                return items

            fin_info = {}

            att_state = {}

            def attention_p1(h, qc, qtr, ktr):
                """Diagonal scores + exps + masks + denominator accs.

                The four trimmed diagonal tiles are computed as two PSUM
                pairs with one exp each (trimmed slots hold stale-PSUM
                garbage that is never read). Their PV matmuls run at the
                END of part2, so the exp/mask chain is hidden behind a
                whole produce item of PE work.
                """
                q0 = qc * CHUNK
                pt_sum = pts_pool.tile([P, CHUNK], fp16, tag="pts")
                diag = []
                for dp in range(2):
                    ps2 = s_psum.tile([P, 2, CHUNK], fp32, tag="pst")
                    for i in range(2):
                        j = 2 * dp + i
                        n0 = j * P
                        nc.tensor.matmul(
                            ps2[:, i, 0:CHUNK - n0],
                            lhsT=ktr[:, (4 * qc + j) * P:(4 * qc + j + 1) * P],
                            rhs=qtr[:, q0 + n0:q0 + CHUNK],
                            start=True, stop=True)
                    pt2 = pt_pool.tile([P, 2, CHUNK], fp16, tag="pt")
                    nc.scalar.activation(pt2, ps2, Exp, scale=SCALE)
                    # both slots' triangle masks in one DVE op
                    nc.vector.tensor_mul(pt2[:, :, 0:P], pt2[:, :, 0:P],
                                         tri2_sb)
                    diag.append(pt2)
                nc.vector.tensor_scalar_add(pt_sum, diag[0][:, 0, :], 0.0)
                for j in (1, 2, 3):
                    n0 = j * P
                    nc.vector.tensor_add(
                        pt_sum[:, n0:], pt_sum[:, n0:],
                        diag[j // 2][:, j % 2, 0:CHUNK - n0])
                att_state[(h, qc)] = (pt_sum, diag)

            def attention_p2(h, qc, qtr, ktr, v_sb, v_col):
                """Off-diagonal pair pipeline + all PV matmuls."""
                q0 = qc * CHUNK
                pt_sum, diag = att_state.pop((h, qc))
                ps_o = o_psum.tile([P, CHUNK], fp32, tag="po")
                n_kt = 4 * qc + 4

                def v_mm(kt, pt_ap, n0):
                    nc.tensor.matmul(
                        ps_o[:, n0:CHUNK],
                        lhsT=v_sb[:, kt, v_col * D:(v_col + 1) * D],
                        rhs=pt_ap, start=(kt == 0), stop=(kt == n_kt - 1))

                for kt in range(0, 4 * qc, 2):
                    ps2 = s_psum.tile([P, 2, CHUNK], fp32, tag="pst")
                    for i in range(2):
                        nc.tensor.matmul(
                            ps2[:, i, :],
                            lhsT=ktr[:, (kt + i) * P:(kt + i + 1) * P],
                            rhs=qtr[:, q0:q0 + CHUNK],
                            start=True, stop=True)
                    pt2 = pt_pool.tile([P, 2, CHUNK], fp16, tag="pt")
                    nc.scalar.activation(pt2, ps2, Exp, scale=SCALE)
                    # pair-internal sum on Pool, += on DVE
                    tmp = tmp_pool.tile([P, CHUNK], fp16, tag="tmp")
                    nc.gpsimd.tensor_add(tmp, pt2[:, 0, :], pt2[:, 1, :])
                    nc.vector.tensor_add(pt_sum, pt_sum, tmp)
                    for i in range(2):
                        v_mm(kt + i, pt2[:, i, :], 0)
                # diagonal PV matmuls last; their exps are long done
                for j in range(4):
                    n0 = j * P
                    v_mm(4 * qc + j, diag[j // 2][:, j % 2, 0:CHUNK - n0], n0)
                fin_info[(h, qc)] = (ps_o, pt_sum)

            def finalize_qc(h, qc):
                """Denominator ones-matmul + reciprocal + normalization."""
                ps_o, pt_sum = fin_info.pop((h, qc))
                q0 = qc * CHUNK
                ps_r = r_psum.tile([P, CHUNK], fp32, tag="pr")
                nc.tensor.matmul(ps_r, lhsT=ones_sb, rhs=pt_sum,
                                 start=True, stop=True)
                rec = rcp_pool.tile([P, CHUNK], fp32, tag="rcp")
                nc.vector.reciprocal_approx_fast(out=rec, in_=ps_r)
                nc.vector.tensor_mul(aot_sb[:, h, q0:q0 + CHUNK], ps_o, rec)

            def attention_items(h):
                qtr, ktr = qk_tiles.pop(h)
                v_sb = v_tiles[h // 2]
                v_col = h % 2
                a1 = [lambda qc=qc: attention_p1(h, qc, qtr, ktr)
                      for qc in range(N_QC)]
                a2 = [lambda qc=qc: attention_p2(h, qc, qtr, ktr, v_sb, v_col)
                      for qc in range(N_QC)]
                return a1, a2

            wo_tiles = {}

            def load_wo(co):
                tiles = []
                for cb in range(HPC):  # contraction chunks == heads
                    w_sb = wo_pool.tile([P, CHUNK], fp16, tag="wo")
                    nc.sync.dma_start(
                        out=w_sb,
                        in_=wo.ap()[cb * P:(cb + 1) * P,
                                    co * CHUNK:(co + 1) * CHUNK])
                    tiles.append(w_sb)
                wo_tiles[co] = tiles

            def ph3_group(co, g, last_group=False):
                """Four t-tiles of the output projection for column group co."""
                tiles = wo_tiles[co]
                for tt in range(4 * g, 4 * g + 4):
                    ps = ph1_psum.tile([P, CHUNK], fp32, tag="ph1")
                    for cb in range(HPC):
                        nc.tensor.matmul(
                            ps, lhsT=aot_sb[:, cb, tt * P:(tt + 1) * P],
                            rhs=tiles[cb],
                            start=(cb == 0), stop=(cb == HPC - 1))
                    yt = y_pool.tile([P, CHUNK], fp16, tag="y")
                    # alternate drains between Act and DVE so neither
                    # engine's in-order stream delays interleaved attention
                    if tt % 2 == 0:
                        nc.vector.tensor_scalar_add(yt, ps, 0.0)
                    else:
                        nc.scalar.copy(yt, ps)
                    if last_group:
                        # spread the final write-backs over three queues so
                        # the drain tail is parallel
                        deng = (nc.sync, nc.scalar, nc.gpsimd, nc.sync)[tt % 4]
                    else:
                        deng = nc.sync if tt % 2 == 0 else nc.scalar
                    deng.dma_start(
                        out=y.ap()[tt * P:(tt + 1) * P,
                                   co * CHUNK:(co + 1) * CHUNK],
                        in_=yt)

            # ---- head steps: produce(h) gate-interleaved with attention(h).
            # attention(h).qc needs only Q-block qc, K-blocks <= qc and
            # V-groups <= qc of its own head, so it lags produce(h) by a
            # couple of items. finalize(h,qc) lags attention(h,qc) by a
            # produce item so the PE's ones-matmul never waits on the DVE
            # accumulation chain.
            pend = []  # delayed finalize carried into the next head

            def flush_pend():
                while pend:
                    pend.pop(0)()

            for h in range(HPC):
                if h + 1 < HPC:
                    load_qk_w(h + 1)
                    if (h + 1) % 2 == 0:
                        load_wv((h + 1) // 2)
                if h == HPC - 2:
                    load_wo(0)
                if h == HPC - 1:
                    load_wo(1)
                p = produce_items(h)     # [Q0..Q3, K0..K3, (V0..V3)]
                a1, a2 = attention_items(h)
                f = [lambda qc=qc, h=h: finalize_qc(h, qc)
                     for qc in range(N_QC)]
                last = h == HPC - 1
                if len(p) == 12:  # even head: Q, K, V blocks
                    # a1[qc] (diag scores+exps) right after K qc; a2[qc]
                    # (PV) a whole V group later so the exp chain is hidden
                    seq = [p[0], flush_pend, p[4], a1[0], p[8], a2[0],
                           p[1], f[0], p[5], a1[1], p[9], a2[1],
                           p[2], f[1], p[6], a1[2], p[10], a2[2],
                           p[3], f[2], p[7], a1[3], p[11], a2[3]]
                    for s in seq:
                        s()
                    pend.append(f[3])
                elif not last:    # odd head: Q, K spacers between a1/a2
                    seq = [p[0], flush_pend, p[4], a1[0], p[1], a2[0],
                           f[0], p[5], a1[1], p[2], a2[1], f[1],
                           p[6], a1[2], p[3], a2[2], f[2],
                           p[7], a1[3], a2[3]]
                    for s in seq:
                        s()
                    pend.append(f[3])
                else:
                    # head 7: phase-3 groups give the Act/DVE chains slack
                    # to drain the attention tail
                    seq = [p[0], flush_pend, p[4], a1[0], p[1], a2[0],
                           f[0], p[5], a1[1], p[2], a2[1], f[1],
                           p[6], a1[2], p[3], a2[2], f[2],
                           p[7], a1[3], a2[3]]
                    for s in seq:
                        s()
                    ph3_group(0, 0)
                    f[3]()
                    for g in range(1, 4):
                        ph3_group(0, g)

            # ---- phase 3: remaining output-projection column groups ----
            for co in range(1, N_QC):
                if co + 1 < N_QC:
                    load_wo(co + 1)
                for g in range(4):
                    ph3_group(co, g, last_group=(co == N_QC - 1 and g == 3))

    nc.finalize()
    return nc


def _host_prep(x, Wq, Wk, Wv, Wo):
    """Build the 8 per-core input maps."""
    perm = np.concatenate([np.arange(0, D, 2), np.arange(1, D, 2)])

    inv_freq = 1.0 / ROPE_THETA ** (np.arange(0, D, 2, dtype=np.float32) / D)
    pos = np.arange(T, dtype=np.float32)
    freqs = np.einsum("i,j->ij", pos, inv_freq)  # [T, 64]
    cosT = np.cos(freqs).T.astype(np.float32)
    sinT = np.sin(freqs).T.astype(np.float32)
    cs = np.concatenate([cosT, cosT], 0).astype(F16)
    sc = np.concatenate([sinT, sinT], 0).astype(F16)
    tri = (np.arange(P)[:, None] <= np.arange(P)[None, :]).astype(F16)

    halves = []
    for g in range(2):
        heads = range(g * HPC, (g + 1) * HPC)
        wq_g = np.stack([Wq[h * D:(h + 1) * D, :][perm, :].T.astype(F16)
                         for h in heads])                       # [8, C, D]
        wk_g = np.stack([Wk[h * D:(h + 1) * D, :][perm, :].T.astype(F16)
                         for h in heads])
        wv_g = np.stack([Wv[(g * HPC + 2 * p) * D:(g * HPC + 2 * p + 2) * D, :]
                         .T.astype(F16) for p in range(HPC // 2)])  # [4, C, 2D]
        wo_g = Wo[:, g * HPC * D:(g + 1) * HPC * D].T.astype(F16)   # [1024, C]
        halves.append(dict(wq=wq_g, wk=wk_g, wv=wv_g, wo=wo_g))

    in_maps = []
    for core in range(N_CORES):
        b, g = core // 2, core % 2
        m = dict(halves[g])
        m["xt"] = np.ascontiguousarray(x[b].T).astype(F16)
        m["cs"] = cs
        m["sc"] = sc
        m["tri"] = tri
        in_maps.append(m)
    return in_maps


def kernel(x, Wq, Wk, Wv, Wo, bo):
    x = np.asarray(x, dtype=np.float32)
    Wq = np.asarray(Wq, dtype=np.float32)
    Wk = np.asarray(Wk, dtype=np.float32)
    Wv = np.asarray(Wv, dtype=np.float32)
    Wo = np.asarray(Wo, dtype=np.float32)
    bo = np.asarray(bo, dtype=np.float32)

    if "nc" not in _CACHED:
        _CACHED["nc"] = build_kernel()
    nc = _CACHED["nc"]

    in_maps = _host_prep(x, Wq, Wk, Wv, Wo)
    res = run_bass_kernel_spmd(nc, in_maps, core_ids=list(range(N_CORES)))

    out = np.empty((B, T, C), np.float32)
    for b in range(B):
        out[b] = (res.results[2 * b]["y"].astype(np.float32)
                  + res.results[2 * b + 1]["y"].astype(np.float32) + bo)
    return out


# revision 14
# speedup vs baseline: 1.0461x; 1.0025x over previous
"""Trainium2 Bass kernel: multi-head attention with RoPE (causal), 8-core SPMD.

Sharding: 8 cores = 4 batches x 2 head-halves (tensor parallel over heads,
data parallel over batch). Each core computes, for its batch b and its 8
heads: QKV projections, RoPE, causal attention, and a partial output
projection. Host sums the two head-half partials per batch and adds the bias.

All TensorEngine math in fp16 with fp32 PSUM accumulation.

v2 changes vs the bf16 baseline:
- softmax denominator no longer uses a per-k-tile ones-matmul on the PE
  (which cost ~58us of PE stream); exp tiles are accumulated into a
  per-(h,qc) pt_sum on DVE/Pool and a single 512-col ones-matmul per
  (h,qc) produces the row sums.
- RoPE has no SBUF-SBUF swap DMAs: with cs=[cos;cos], sc=[sin;sin] the
  cross-partition combine is done directly by DVE tensor ops whose PSUM
  operand may use a different base partition than the SBUF operand
  (verifier only forbids misaligned bases when BOTH inputs are in SBUF).
- fp16 instead of bf16 (same PE rate, 2x DVE mode, more mantissa).
- head-0 weight loads are chunked so the first matmul starts ~2us in.
- V-projection PSUM drains moved to the Act engine (same act table as
  Exp, so no table-switch cost).

Schedule: attention(h) is interleaved with produce(h) at qc/block
granularity; the per-(h,qc) finalize (ones-matmul + reciprocal + norm)
is delayed a couple of items so the PE never waits on the DVE add chain.

DMA queues: weights + consts + xt qc1-3 + y write-back on sync; cs/sc and
half the y write-back on scalar; xt qc0 on gpsimd.
"""

import sys

for _p in ("/opt/trn_rl_repo",):
    if _p not in sys.path:
        sys.path.insert(0, _p)

import numpy as np

import concourse.bass as bass
import concourse.bacc as bacc
import concourse.mybir as mybir
import concourse.tile as tile
from concourse.bass_utils import run_bass_kernel_spmd

F16 = np.float16

B, T, C = 4, 2048, 2048
H = 16
D = C // H  # 128
ROPE_THETA = 1000000.0
N_CORES = 8
HPC = H // 2          # heads per core (8)
P = 128               # partitions
CHUNK = 512           # moving free dim per matmul
N_CC = C // P         # 16 contraction chunks
N_QC = T // CHUNK     # 4 q-chunks
N_KT = T // P         # 16 k-tiles
SCALE = 1.0 / float(np.sqrt(D))
NWC = 4               # weight-load chunks per projection matrix

_CACHED = {}


def build_kernel():
    """Build the SPMD Bass program (identical on all 8 cores)."""
    fp32 = mybir.dt.float32
    fp16 = mybir.dt.float16

    nc = bacc.Bacc("TRN2", target_bir_lowering=False, debug=False,
                   num_devices=N_CORES)

    # Per-core DRAM inputs (fp16 unless noted)
    xt = nc.dram_tensor("xt", [C, T], fp16, kind="ExternalInput")          # x[b].T
    wq = nc.dram_tensor("wq", [HPC, C, D], fp16, kind="ExternalInput")     # Wq_h.T (perm'd)
    wk = nc.dram_tensor("wk", [HPC, C, D], fp16, kind="ExternalInput")
    wv = nc.dram_tensor("wv", [HPC // 2, C, 2 * D], fp16, kind="ExternalInput")  # head pairs
    wo = nc.dram_tensor("wo", [HPC * D, C], fp16, kind="ExternalInput")    # WoT slice
    cs = nc.dram_tensor("cs", [P, T], fp16, kind="ExternalInput")          # [cosT;cosT]
    sc = nc.dram_tensor("sc", [P, T], fp16, kind="ExternalInput")          # [sinT;sinT]
    tri = nc.dram_tensor("tri", [P, P], fp16, kind="ExternalInput")        # k<=q mask
    y = nc.dram_tensor("y", [T, C], fp16, kind="ExternalOutput")

    Exp = mybir.ActivationFunctionType.Exp

    with tile.TileContext(nc) as tc:
        with (
            tc.tile_pool(name="const", bufs=1) as const_pool,
            tc.tile_pool(name="xtp", bufs=1) as xt_pool,
            tc.tile_pool(name="wqk", bufs=4) as wqk_pool,
            tc.tile_pool(name="wvp", bufs=2) as wv_pool,
            tc.tile_pool(name="bt", bufs=2) as b_pool,
            tc.tile_pool(name="qktr", bufs=2) as qk_pool,
            tc.tile_pool(name="vsb", bufs=2) as v_pool,
            tc.tile_pool(name="pt", bufs=4) as pt_pool,
            tc.tile_pool(name="pts", bufs=2) as pts_pool,
            tc.tile_pool(name="tmp", bufs=2) as tmp_pool,
            tc.tile_pool(name="rcp", bufs=2) as rcp_pool,
            tc.tile_pool(name="aot", bufs=1) as aot_pool,
            tc.tile_pool(name="wop", bufs=16) as wo_pool,
            tc.tile_pool(name="yst", bufs=2) as y_pool,
            tc.tile_pool(name="ph1", bufs=2, space="PSUM") as ph1_psum,
            tc.tile_pool(name="pst", bufs=2, space="PSUM") as s_psum,
            tc.tile_pool(name="po", bufs=1, space="PSUM") as o_psum,
            tc.tile_pool(name="pr", bufs=1, space="PSUM") as r_psum,
        ):
            # ---- weight loads: chunked on the sync queue ----
            qk_w = {}

            def load_qk_w(h):
                wq_r = wq.ap()[h].rearrange("(cc p) d -> p cc d", p=P)
                wk_r = wk.ap()[h].rearrange("(cc p) d -> p cc d", p=P)
                tiles = []
                for src in (wq_r, wk_r):
                    for ch in range(NWC):
                        t_sb = wqk_pool.tile([P, N_CC // NWC, D], fp16,
                                             tag=f"wqk{ch}")
                        nc.sync.dma_start(
                            out=t_sb,
                            in_=src[:, ch * (N_CC // NWC):(ch + 1) * (N_CC // NWC), :])
                        tiles.append(t_sb)
                qk_w[h] = tiles  # [wq c0..c3, wk c0..c3]

            wv_w = {}

            def load_wv(pair):
                wv_r = wv.ap()[pair].rearrange("(cc p) d -> p cc d", p=P)
                tiles = []
                for ch in range(NWC):
                    t_sb = wv_pool.tile([P, N_CC // NWC, 2 * D], fp16,
                                        tag=f"wv{ch}")
                    nc.sync.dma_start(
                        out=t_sb,
                        in_=wv_r[:, ch * (N_CC // NWC):(ch + 1) * (N_CC // NWC), :])
                    tiles.append(t_sb)
                wv_w[pair] = tiles

            load_qk_w(0)

            # ---- constants (scalar queue, parallel with sync weights) ----
            cs_sb = const_pool.tile([P, T], fp16)
            sc_sb = const_pool.tile([P, T], fp16)
            tri2_sb = const_pool.tile([P, 2, P], fp16)
            ones_sb = const_pool.tile([P, P], fp16)
            nc.scalar.dma_start(out=cs_sb, in_=cs.ap())
            nc.scalar.dma_start(out=sc_sb, in_=sc.ap())
            nc.scalar.dma_start(out=tri2_sb[:, 0, :], in_=tri.ap())
            nc.scalar.dma_start(out=tri2_sb[:, 1, :], in_=tri.ap())
            load_wv(0)
            nc.vector.memset(ones_sb, 1.0)

            # ---- x^T load: 64 chunk tiles ----
            xt_t = {}
            xt_r = xt.ap().rearrange("(cc p) t -> p cc t", p=P)
            for qc in range(N_QC):
                for cc in range(N_CC):
                    t_sb = xt_pool.tile([P, CHUNK], fp16, tag=f"xt{cc}_{qc}")
                    # qc0 on the gpsimd queue so head 0 starts fast; the
                    # rest stream on sync behind the head-0 weights.
                    eng = nc.gpsimd if qc == 0 else nc.sync
                    eng.dma_start(
                        out=t_sb,
                        in_=xt_r[:, cc, qc * CHUNK:(qc + 1) * CHUNK])
                    xt_t[(cc, qc)] = t_sb

            aot_sb = aot_pool.tile([P, HPC, T], fp16)  # attn-out^T, all heads

            # ---- work items ----
            def qk_block(w_chunks, out_sb, qc):
                """One 512-wide projection block with RoPE applied.

                RoPE without swap DMAs: the Act engine drains the PSUM
                block twice -- straight (qs=[e;o]) and partition-swapped
                (qw=[o;e]) -- so every DVE op below is an aligned all-SBUF
                fp16 op that runs in the 2x DVE mode:
                  qs *= cs ([cos;cos])  -> [e*cos; o*cos]
                  qw *= sc ([sin;sin])  -> [o*sin; e*sin]
                  re[0:64]   = qs[0:64]   - qw[0:64]
                  ro[64:128] = qs[64:128] + qw[64:128]
                This also releases the PSUM tile after the (fast) Act
                drains instead of at the end of the DVE chain.
                """
                ps = ph1_psum.tile([P, CHUNK], fp32, tag="ph1")
                for cc in range(N_CC):
                    nc.tensor.matmul(
                        ps, lhsT=w_chunks[cc // NWC][:, cc % NWC, :],
                        rhs=xt_t[(cc, qc)],
                        start=(cc == 0), stop=(cc == N_CC - 1))
                sl = slice(qc * CHUNK, (qc + 1) * CHUNK)
                qs = b_pool.tile([P, CHUNK], fp16, tag="qs")
                qw = b_pool.tile([P, CHUNK], fp16, tag="qw")
                nc.scalar.copy(qs, ps)
                nc.scalar.copy(qw[0:64, :], ps[64:128, :])
                nc.scalar.copy(qw[64:128, :], ps[0:64, :])
                nc.vector.tensor_mul(qs, qs, cs_sb[:, sl])
                nc.vector.tensor_mul(qw, qw, sc_sb[:, sl])
                nc.vector.tensor_sub(out_sb[0:64, sl], qs[0:64, :],
                                     qw[0:64, :])
                nc.vector.tensor_add(out_sb[64:128, sl], qs[64:128, :],
                                     qw[64:128, :])

            def v_block(w_chunks, v_sb, tt0):
                """Four t-tiles of the V projection for one head pair."""
                for tt in range(tt0, tt0 + 4):
                    ps = ph1_psum.tile([P, CHUNK], fp32, tag="ph1")
                    psv = ps[:, 0:2 * D]
                    t0 = (tt % 4) * P
                    for cc in range(N_CC):
                        nc.tensor.matmul(
                            psv, lhsT=xt_t[(cc, tt // 4)][:, t0:t0 + P],
                            rhs=w_chunks[cc // NWC][:, cc % NWC, :],
                            start=(cc == 0), stop=(cc == N_CC - 1))
                    # alternate drains Act/DVE so a burst on either engine
                    # doesn't hold the ph1 ring
                    if tt % 2 == 0:
                        nc.vector.tensor_scalar_add(v_sb[:, tt, :], psv, 0.0)
                    else:
                        nc.scalar.copy(v_sb[:, tt, :], psv)

            v_tiles = {}
            qk_tiles = {}

            def produce_items(h):
                w_tiles = qk_w.pop(h)
                qtr = qk_pool.tile([P, T], fp16, tag="wqtr", name=f"qtr{h}")
                ktr = qk_pool.tile([P, T], fp16, tag="wktr", name=f"ktr{h}")
                qk_tiles[h] = (qtr, ktr)
                items = []
                for qc in range(N_QC):
                    items.append(lambda qc=qc: qk_block(w_tiles[:NWC], qtr, qc))
                for qc in range(N_QC):
                    items.append(lambda qc=qc: qk_block(w_tiles[NWC:], ktr, qc))
                if h % 2 == 0:
                    w_v = wv_w.pop(h // 2)
                    v_sb = v_pool.tile([P, N_KT, 2 * D], fp16, tag="vsb",
                                       name=f"vsb{h // 2}")
                    v_tiles[h // 2] = v_sb
                    for g in range(4):
                        items.append(lambda g=g, w=w_v, v=v_sb: v_block(w, v, 4 * g))
                return items

            fin_info = {}

            att_state = {}

            def attention_p1(h, qc, qtr, ktr):
                """Diagonal scores + exps + masks + denominator accs.

                The four trimmed diagonal tiles are computed as two PSUM
                pairs with one exp each (trimmed slots hold stale-PSUM
                garbage that is never read). Their PV matmuls run at the
                END of part2, so the exp/mask chain is hidden behind a
                whole produce item of PE work.
                """
                q0 = qc * CHUNK
                pt_sum = pts_pool.tile([P, CHUNK], fp16, tag="pts")
                diag = []
                for dp in range(2):
                    ps2 = s_psum.tile([P, 2, CHUNK], fp32, tag="pst")
                    for i in range(2):
                        j = 2 * dp + i
                        n0 = j * P
                        nc.tensor.matmul(
                            ps2[:, i, 0:CHUNK - n0],
                            lhsT=ktr[:, (4 * qc + j) * P:(4 * qc + j + 1) * P],
                            rhs=qtr[:, q0 + n0:q0 + CHUNK],
                            start=True, stop=True)
                    pt2 = pt_pool.tile([P, 2, CHUNK], fp16, tag="pt")
                    nc.scalar.activation(pt2, ps2, Exp, scale=SCALE)
                    # both slots' triangle masks in one DVE op
                    nc.vector.tensor_mul(pt2[:, :, 0:P], pt2[:, :, 0:P],
                                         tri2_sb)
                    diag.append(pt2)
                nc.vector.tensor_scalar_add(pt_sum, diag[0][:, 0, :], 0.0)
                for j in (1, 2, 3):
                    n0 = j * P
                    nc.vector.tensor_add(
                        pt_sum[:, n0:], pt_sum[:, n0:],
                        diag[j // 2][:, j % 2, 0:CHUNK - n0])
                att_state[(h, qc)] = (pt_sum, diag)

            def attention_p2(h, qc, qtr, ktr, v_sb, v_col):
                """Off-diagonal pair pipeline + all PV matmuls."""
                q0 = qc * CHUNK
                pt_sum, diag = att_state.pop((h, qc))
                ps_o = o_psum.tile([P, CHUNK], fp32, tag="po")
                n_kt = 4 * qc + 4

                def v_mm(kt, pt_ap, n0):
                    nc.tensor.matmul(
                        ps_o[:, n0:CHUNK],
                        lhsT=v_sb[:, kt, v_col * D:(v_col + 1) * D],
                        rhs=pt_ap, start=(kt == 0), stop=(kt == n_kt - 1))

                for kt in range(0, 4 * qc, 2):
                    ps2 = s_psum.tile([P, 2, CHUNK], fp32, tag="pst")
                    for i in range(2):
                        nc.tensor.matmul(
                            ps2[:, i, :],
                            lhsT=ktr[:, (kt + i) * P:(kt + i + 1) * P],
                            rhs=qtr[:, q0:q0 + CHUNK],
                            start=True, stop=True)
                    pt2 = pt_pool.tile([P, 2, CHUNK], fp16, tag="pt")
                    nc.scalar.activation(pt2, ps2, Exp, scale=SCALE)
                    # pair-internal sum on Pool, += on DVE
                    tmp = tmp_pool.tile([P, CHUNK], fp16, tag="tmp")
                    nc.gpsimd.tensor_add(tmp, pt2[:, 0, :], pt2[:, 1, :])
                    nc.vector.tensor_add(pt_sum, pt_sum, tmp)
                    for i in range(2):
                        v_mm(kt + i, pt2[:, i, :], 0)
                # diagonal PV matmuls last; their exps are long done
                for j in range(4):
                    n0 = j * P
                    v_mm(4 * qc + j, diag[j // 2][:, j % 2, 0:CHUNK - n0], n0)
                fin_info[(h, qc)] = (ps_o, pt_sum)

            def finalize_qc(h, qc):
                """Denominator ones-matmul + reciprocal + normalization."""
                ps_o, pt_sum = fin_info.pop((h, qc))
                q0 = qc * CHUNK
                ps_r = r_psum.tile([P, CHUNK], fp32, tag="pr")
                nc.tensor.matmul(ps_r, lhsT=ones_sb, rhs=pt_sum,
                                 start=True, stop=True)
                rec = rcp_pool.tile([P, CHUNK], fp32, tag="rcp")
                nc.vector.reciprocal_approx_fast(out=rec, in_=ps_r)
                nc.vector.tensor_mul(aot_sb[:, h, q0:q0 + CHUNK], ps_o, rec)

            def attention_items(h):
                qtr, ktr = qk_tiles.pop(h)
                v_sb = v_tiles[h // 2]
                v_col = h % 2
                a1 = [lambda qc=qc: attention_p1(h, qc, qtr, ktr)
                      for qc in range(N_QC)]
                a2 = [lambda qc=qc: attention_p2(h, qc, qtr, ktr, v_sb, v_col)
                      for qc in range(N_QC)]
                return a1, a2

            wo_tiles = {}

            def load_wo(co):
                tiles = []
                for cb in range(HPC):  # contraction chunks == heads
                    w_sb = wo_pool.tile([P, CHUNK], fp16, tag="wo")
                    nc.sync.dma_start(
                        out=w_sb,
                        in_=wo.ap()[cb * P:(cb + 1) * P,
                                    co * CHUNK:(co + 1) * CHUNK])
                    tiles.append(w_sb)
                wo_tiles[co] = tiles

            def ph3_group(co, g, last_group=False):
                """Four t-tiles of the output projection for column group co."""
                tiles = wo_tiles[co]
                for tt in range(4 * g, 4 * g + 4):
                    ps = ph1_psum.tile([P, CHUNK], fp32, tag="ph1")
                    for cb in range(HPC):
                        nc.tensor.matmul(
                            ps, lhsT=aot_sb[:, cb, tt * P:(tt + 1) * P],
                            rhs=tiles[cb],
                            start=(cb == 0), stop=(cb == HPC - 1))
                    yt = y_pool.tile([P, CHUNK], fp16, tag="y")
                    # alternate drains between Act and DVE so neither
                    # engine's in-order stream delays interleaved attention
                    if tt % 2 == 0:
                        nc.vector.tensor_scalar_add(yt, ps, 0.0)
                    else:
                        nc.scalar.copy(yt, ps)
                    if last_group:
                        # spread the final write-backs over three queues so
                        # the drain tail is parallel
                        deng = (nc.sync, nc.scalar, nc.gpsimd, nc.sync)[tt % 4]
                    else:
                        deng = nc.sync if tt % 2 == 0 else nc.scalar
                    deng.dma_start(
                        out=y.ap()[tt * P:(tt + 1) * P,
                                   co * CHUNK:(co + 1) * CHUNK],
                        in_=yt)

            # ---- head steps: produce(h) gate-interleaved with attention(h).
            # attention(h).qc needs only Q-block qc, K-blocks <= qc and
            # V-groups <= qc of its own head, so it lags produce(h) by a
            # couple of items. finalize(h,qc) lags attention(h,qc) by a
            # produce item so the PE's ones-matmul never waits on the DVE
            # accumulation chain.
            pend = []  # delayed finalize carried into the next head

            def flush_pend():
                while pend:
                    pend.pop(0)()

            for h in range(HPC):
                if h + 1 < HPC:
                    load_qk_w(h + 1)
                    if (h + 1) % 2 == 0:
                        load_wv((h + 1) // 2)
                if h == HPC - 2:
                    load_wo(0)
                if h == HPC - 1:
                    load_wo(1)
                p = produce_items(h)     # [Q0..Q3, K0..K3, (V0..V3)]
                a1, a2 = attention_items(h)
                f = [lambda qc=qc, h=h: finalize_qc(h, qc)
                     for qc in range(N_QC)]
                last = h == HPC - 1
                if len(p) == 12:  # even head: Q, K, V blocks
                    # a1[qc] (diag scores+exps) right after K qc; a2[qc]
                    # (PV) a whole V group later so the exp chain is hidden
                    seq = [p[0], flush_pend, p[4], a1[0], p[8], a2[0],
                           p[1], f[0], p[5], a1[1], p[9], a2[1],
                           p[2], f[1], p[6], a1[2], p[10], a2[2],
                           p[3], f[2], p[7], a1[3], p[11], a2[3]]
                    for s in seq:
                        s()
                    pend.append(f[3])
                elif not last:    # odd head: Q, K spacers between a2/f
                    seq = [p[0], flush_pend, p[4], a1[0], p[1], a2[0],
                           p[5], f[0], a1[1], p[2], a2[1], p[6], f[1],
                           a1[2], p[3], a2[2], p[7], f[2],
                           a1[3], a2[3]]
                    for s in seq:
                        s()
                    pend.append(f[3])
                else:
                    # head 7: phase-3 groups give the Act/DVE chains slack
                    # to drain the attention tail
                    seq = [p[0], flush_pend, p[4], a1[0], p[1], a2[0],
                           p[5], f[0], a1[1], p[2], a2[1], p[6], f[1],
                           a1[2], p[3], a2[2], p[7], f[2],
                           a1[3], a2[3]]
                    for s in seq:
                        s()
                    ph3_group(0, 0)
                    f[3]()
                    for g in range(1, 4):
                        ph3_group(0, g)

            # ---- phase 3: remaining output-projection column groups ----
            for co in range(1, N_QC):
                if co + 1 < N_QC:
                    load_wo(co + 1)
                for g in range(4):
                    ph3_group(co, g, last_group=(co == N_QC - 1 and g == 3))

    nc.finalize()
    return nc


def _host_prep(x, Wq, Wk, Wv, Wo):
    """Build the 8 per-core input maps."""
    perm = np.concatenate([np.arange(0, D, 2), np.arange(1, D, 2)])

    inv_freq = 1.0 / ROPE_THETA ** (np.arange(0, D, 2, dtype=np.float32) / D)
    pos = np.arange(T, dtype=np.float32)
    freqs = np.einsum("i,j->ij", pos, inv_freq)  # [T, 64]
    cosT = np.cos(freqs).T.astype(np.float32)
    sinT = np.sin(freqs).T.astype(np.float32)
    cs = np.concatenate([cosT, cosT], 0).astype(F16)
    sc = np.concatenate([sinT, sinT], 0).astype(F16)
    tri = (np.arange(P)[:, None] <= np.arange(P)[None, :]).astype(F16)

    halves = []
    for g in range(2):
        heads = range(g * HPC, (g + 1) * HPC)
        wq_g = np.stack([Wq[h * D:(h + 1) * D, :][perm, :].T.astype(F16)
                         for h in heads])                       # [8, C, D]
        wk_g = np.stack([Wk[h * D:(h + 1) * D, :][perm, :].T.astype(F16)
                         for h in heads])
        wv_g = np.stack([Wv[(g * HPC + 2 * p) * D:(g * HPC + 2 * p + 2) * D, :]
                         .T.astype(F16) for p in range(HPC // 2)])  # [4, C, 2D]
        wo_g = Wo[:, g * HPC * D:(g + 1) * HPC * D].T.astype(F16)   # [1024, C]
        halves.append(dict(wq=wq_g, wk=wk_g, wv=wv_g, wo=wo_g))

    in_maps = []
    for core in range(N_CORES):
        b, g = core // 2, core % 2
        m = dict(halves[g])
        m["xt"] = np.ascontiguousarray(x[b].T).astype(F16)
        m["cs"] = cs
        m["sc"] = sc
        m["tri"] = tri
        in_maps.append(m)
    return in_maps


def kernel(x, Wq, Wk, Wv, Wo, bo):
    x = np.asarray(x, dtype=np.float32)
    Wq = np.asarray(Wq, dtype=np.float32)
    Wk = np.asarray(Wk, dtype=np.float32)
    Wv = np.asarray(Wv, dtype=np.float32)
    Wo = np.asarray(Wo, dtype=np.float32)
    bo = np.asarray(bo, dtype=np.float32)

    if "nc" not in _CACHED:
        _CACHED["nc"] = build_kernel()
    nc = _CACHED["nc"]

    in_maps = _host_prep(x, Wq, Wk, Wv, Wo)
    res = run_bass_kernel_spmd(nc, in_maps, core_ids=list(range(N_CORES)))

    out = np.empty((B, T, C), np.float32)
    for b in range(B):
        out[b] = (res.results[2 * b]["y"].astype(np.float32)
                  + res.results[2 * b + 1]["y"].astype(np.float32) + bo)
    return out
